# revision 23
# baseline (speedup 1.0000x reference)
"""Pre-LN transformer block (B=2,T=2048,C=1024,H=16) on 8 TRN2 NeuronCores.

Two SPMD launches:
  L1: tensor-parallel over heads (2 heads/core) - LN1 stats on-chip
      (replicated, folded algebraically into the QKV matmuls), causal
      attention with on-chip softmax, normalized attn^T output per core.
  L2: parallel over token rows (512 rows/core) - output projection +
      residual, LN2, FFN (relu) + residual.
Host work between launches is just resharding: slicing/concat and dtype
casts. All matmuls run in bf16 with fp32 PSUM accumulation; residuals
are carried in fp32.
"""
"""Transformer block on 8 TRN2 cores: L1 = head-parallel attention, L2 = row-parallel proj+FFN."""
import contextlib
import numpy as np
import ml_dtypes

import concourse.bass as bass
import concourse.mybir as mybir
import concourse.tile as tile
from concourse import bacc
from concourse.masks import make_identity

bf16 = ml_dtypes.bfloat16
FP32 = mybir.dt.float32
BF16 = mybir.dt.bfloat16
AF = mybir.ActivationFunctionType

B, T, C, H = 2, 2048, 1024, 16
HS = C // H          # 64
NCORES = 8
HPC = H // NCORES    # 2 heads per core
TOK = B * T          # 4096
EPS = 1e-5
CT = C // 128        # 8 c-tiles
NCH = TOK // 512     # 8 512-col chunks of token axis
QB = 512             # query block
ROWS = TOK // NCORES # 512 rows per core in L2
HID = 4 * C          # 4096
HT = HID // 128      # 32 hidden tiles
MT = ROWS // 128     # 4 token tiles in L2


def build_l1(use_beta=True):
    """L1: head-parallel attention (2 heads/core).

    Phase 1 (per 512-token chunk, interleaved so PE stays dense while xT
    streams in): LN1 stats via 1/C-folded ones-matmuls (born broadcast),
    QKV on raw xT with the LN fold applied at drain, V' transpose tiles.
    Phase 2: causal attention with k-tile-PAIRED exp (one ACT instruction
    per [128,1024] PSUM supertile spanning 2 banks) and 65-wide attn-V
    (ones column = softmax denominator). Output is UNNORMALIZED attn plus
    denominators [130, TOK] fp32; the host divides (free) before L2.
    Batch-0 attention is emitted interleaved with batch-1's phase 1 so the
    ACT-bound exp overlaps the PE-bound QKV work.
    """
    nc = bacc.Bacc("TRN2", target_bir_lowering=False, debug=False, num_devices=NCORES)
    xt_d = nc.dram_tensor("xt", [C, TOK], BF16, kind="ExternalInput").ap()
    wq_d = nc.dram_tensor("wq", [C, 128], BF16, kind="ExternalInput").ap()
    wk_d = nc.dram_tensor("wk", [C, 128], BF16, kind="ExternalInput").ap()
    wv_d = nc.dram_tensor("wv", [C, 128], BF16, kind="ExternalInput").ap()
    # negated column sums of wq/wk/wv and W.T @ beta1, all [128,1] fp32
    nws_d = nc.dram_tensor("nws", [128, 3], FP32, kind="ExternalInput").ap()
    wb_d = nc.dram_tensor("wb", [128, 3], FP32, kind="ExternalInput").ap()
    tri_d = nc.dram_tensor("tri", [128, 128], BF16, kind="ExternalInput").ap()
    # rows 0-63: h0 attn (unnormalized), 64: h0 denom, 65-128: h1, 129: h1 denom
    out_d = nc.dram_tensor("attn_out", [130, TOK], FP32, kind="ExternalOutput").ap()

    with tile.TileContext(nc) as tc, contextlib.ExitStack() as ctx:
        consts = ctx.enter_context(tc.tile_pool(name="consts", bufs=1))
        hpool = ctx.enter_context(tc.tile_pool(name="hT", bufs=1))
        stats = ctx.enter_context(tc.tile_pool(name="stats", bufs=1))
        tmp = ctx.enter_context(tc.tile_pool(name="tmp", bufs=3))
        qkv = ctx.enter_context(tc.tile_pool(name="qkv", bufs=1))
        probs_p = ctx.enter_context(tc.tile_pool(name="probs", bufs=6))
        attn_sb_p = ctx.enter_context(tc.tile_pool(name="attn_sb", bufs=4))
        # PSUM budget (8 banks): sup0 + sup1 ([128,1024] = 2 banks each),
        # pa0 + pa1 (1 each), mm (1), tr (1)
        ps_sup = ctx.enter_context(tc.tile_pool(name="ps_sup", bufs=1, space="PSUM"))
        ps_acc = ctx.enter_context(tc.tile_pool(name="ps_acc", bufs=1, space="PSUM"))
        ps_mm = ctx.enter_context(tc.tile_pool(name="ps_mm", bufs=1, space="PSUM"))
        ps_tr = ctx.enter_context(tc.tile_pool(name="ps_tr", bufs=1, space="PSUM"))

        # ---- xT as ONE packed tile [128, CT, TOK]: chunk 0 in a single DMA
        # (shortest time-to-first-compute), then two big remainder DMAs.
        # Issued before everything else; far half rides the gpsimd SW-DGE.
        xbig = hpool.tile([128, CT, TOK], BF16, tag="xbig")
        xt_r = xt_d.rearrange("(a p) m -> p a m", p=128)
        nc.sync.dma_start(out=xbig[:, :, 0:512], in_=xt_r[:, :, 0:512])
        nc.sync.dma_start(out=xbig[:, :, 512:2048], in_=xt_r[:, :, 512:2048])
        nc.gpsimd.dma_start(out=xbig[:, :, 2048:3072], in_=xt_r[:, :, 2048:3072])
        nc.gpsimd.dma_start(out=xbig[:, :, 3072:4096], in_=xt_r[:, :, 3072:4096])
        xts = [xbig[:, ci, :] for ci in range(CT)]

        # ---- constants ----
        ones_sb = consts.tile([128, 128], BF16)
        nc.vector.memset(ones_sb, 1.0 / C)
        eps_sb = consts.tile([128, 1], FP32)
        nc.vector.memset(eps_sb, EPS)
        ident = consts.tile([128, 128], BF16)
        make_identity(nc, ident)

        wq_sb = consts.tile([128, CT, 128], BF16)
        nc.sync.dma_start(out=wq_sb, in_=wq_d.rearrange("(a p) m -> p a m", p=128))
        wk_sb = consts.tile([128, CT, 128], BF16)
        nc.sync.dma_start(out=wk_sb, in_=wk_d.rearrange("(a p) m -> p a m", p=128))
        wv_sb = consts.tile([128, CT, 128], BF16)
        nc.sync.dma_start(out=wv_sb, in_=wv_d.rearrange("(a p) m -> p a m", p=128))
        nws_sb = consts.tile([128, 3], FP32)
        nc.sync.dma_start(out=nws_sb, in_=nws_d)
        wb_sb = consts.tile([128, 3], FP32)
        nc.sync.dma_start(out=wb_sb, in_=wb_d)
        tri_sb = consts.tile([128, 128], BF16)
        nc.sync.dma_start(out=tri_sb, in_=tri_d)

        # PE warm-up spin: flips HAM to 8/8 while inputs stream (each spin is
        # ~0.5us cold incl. the LDW churn; don't overshoot or chunk 0 waits)
        warm_ps = ps_mm.tile([128, 512], FP32, tag="mm")
        for _ in range(12):
            nc.tensor.matmul(warm_ps[:, 0:128], ones_sb, ones_sb[:, 0:128],
                             start=True, stop=True)

        rstd_b = stats.tile([128, TOK], BF16, tag="rstd_b")
        murstd_b = stats.tile([128, TOK], BF16, tag="murstd_b")
        qt_sb = qkv.tile([128, TOK], BF16, tag="qt")
        kt_sb = qkv.tile([128, TOK], BF16, tag="kt")
        vt_sb = qkv.tile([128, TOK], BF16, tag="vt")
        vts = []
        for tt in range(TOK // 128):
            vt = qkv.tile([128, 130], BF16, tag=f"v{tt}")
            vts.append(vt)
        mu_bat, var_bat = {}, {}
        for h in range(2):
            mb = stats.tile([128, 2048], FP32, tag=f"muB{h}")
            vb = stats.tile([128, 2048], FP32, tag=f"varB{h}")
            mu_bat[h], var_bat[h] = mb, vb

        def stats_qkv_chunk(j):
            """stats matmuls + derive-to-var (no ACT Ln yet) + raw QKV drains."""
            sl = slice(j * 512, (j + 1) * 512)
            bsl = slice((j % 4) * 512, (j % 4 + 1) * 512)
            mu, var = mu_bat[j // 4][:, bsl], var_bat[j // 4][:, bsl]
            st = ps_sup.tile([128, 1024], FP32, tag=f"sup{j % 2}")
            for ci in range(CT):
                sq = tmp.tile([128, 512], BF16, tag="sq")
                nc.vector.tensor_mul(sq, xts[ci][:, sl], xts[ci][:, sl])
                nc.tensor.matmul(st[:, 0:512], ones_sb, xts[ci][:, sl],
                                 start=(ci == 0), stop=(ci == CT - 1),
                                 skip_group_check=True)
                nc.tensor.matmul(st[:, 512:1024], ones_sb, sq,
                                 start=(ci == 0), stop=(ci == CT - 1),
                                 skip_group_check=True)
            nc.vector.tensor_copy(mu, st[:, 0:512])
            mu2 = tmp.tile([128, 512], FP32, tag="mu2")
            nc.vector.tensor_mul(mu2, mu, mu)
            nc.vector.tensor_sub(var, st[:, 512:1024], mu2)
            # QKV raw drains (fold applied after the batched Ln/Exp pass)
            for wsb, idx, tsb in ((wq_sb, 0, qt_sb), (wk_sb, 1, kt_sb), (wv_sb, 2, vt_sb)):
                pp = ps_mm.tile([128, 512], FP32, tag="mm")
                for ci in range(CT):
                    nc.tensor.matmul(pp, wsb[:, ci, :], xts[ci][:, sl],
                                     start=(ci == 0), stop=(ci == CT - 1))
                nc.scalar.copy(out=tsb[:, sl], in_=pp)

        def derive_batch(h):
            """rstd for 4 chunks in ONE wide Ln and ONE wide Exp instruction —
            single instructions can't be interleaved by the scheduler, so the
            ACT table set switches exactly twice per batch."""
            hsl = slice(h * 2048, (h + 1) * 2048)
            nc.scalar.activation(out=var_bat[h], in_=var_bat[h], func=AF.Ln,
                                 bias=eps_sb, scale=1.0)
            nc.scalar.activation(out=rstd_b[:, hsl], in_=var_bat[h], func=AF.Exp,
                                 scale=-0.5)
            nc.vector.tensor_mul(murstd_b[:, hsl], mu_bat[h], rstd_b[:, hsl])
            # keep-warm spins anchored on the just-produced stats so the
            # scheduler can't hoist them out of the derive bubble (fresh mm
            # tile so the pool serializes them against the last QKV drain)
            wsp = ps_mm.tile([128, 512], FP32, tag="mm")
            for _ in range(2):
                nc.tensor.matmul(wsp[:, 0:128], ones_sb,
                                 murstd_b[:, h * 2048:h * 2048 + 128],
                                 start=True, stop=True, skip_group_check=True)

        def fold_chunk(j):
            """apply the LN fold to the raw QKV drains, then build V' tiles."""
            sl = slice(j * 512, (j + 1) * 512)
            for idx, tsb in ((0, qt_sb), (1, kt_sb), (2, vt_sb)):
                nc.vector.tensor_mul(tsb[:, sl], tsb[:, sl], rstd_b[:, sl])
                nc.vector.scalar_tensor_tensor(
                    tsb[:, sl], murstd_b[:, sl], nws_sb[:, idx:idx + 1], tsb[:, sl],
                    op0=mybir.AluOpType.mult, op1=mybir.AluOpType.add)
                if use_beta:
                    nc.vector.tensor_scalar_add(tsb[:, sl], tsb[:, sl],
                                                wb_sb[:, idx:idx + 1])
            # V' tiles [128,130] = [v_h0 | ones | v_h1 | ones]
            for tt in range(j * 4, j * 4 + 4):
                vt = vts[tt]
                ptv = ps_tr.tile([128, 128], BF16, tag="tr")
                nc.tensor.transpose(ptv, vt_sb[:, tt * 128:(tt + 1) * 128], ident)
                nc.vector.tensor_copy(
                    vt.rearrange("p (g c) -> p g c", g=2)[:, :, 0:64],
                    ptv.rearrange("p (g c) -> p g c", g=2))
                nc.vector.memset(vt[:, 64:65], 1.0)
                nc.vector.memset(vt[:, 129:130], 1.0)

        # ---- attention generators (paired k-tiles -> one exp per 2 banks) ----
        scale = C ** -0.5
        # HAM keep-warm filler target; set once phase 1 stops using the mm bank
        fill_cell = []

        def attn_group(b, hl):
            hsl = slice(hl * 64, (hl + 1) * 64)
            vcol = slice(hl * 65, hl * 65 + 65)
            pend = []

            def _flush(item):
                pa_, q0_, koff_, pr_ap_, c0_, st_, sp_ = item
                nc.tensor.matmul(pa_[:, c0_:] if c0_ else pa_,
                                 vts[koff_ // 128][:, vcol], pr_ap_,
                                 start=st_, stop=sp_, skip_group_check=True)
                if sp_:
                    asb = attn_sb_p.tile([65, 512], FP32, tag="asb")
                    nc.vector.tensor_copy(asb, pa_)
                    nc.sync.dma_start(
                        out=out_d[hl * 65:hl * 65 + 65, q0_:q0_ + QB], in_=asb)

            for j in range(T // QB):
                q0 = b * T + j * QB
                pa = ps_acc.tile([65, 512], FP32, tag=f"pa{hl}")
                nkt = 4 * (j + 1)
                for p in range(nkt // 2):
                    sup = ps_sup.tile([128, 1024], FP32, tag=f"sup{hl}")
                    pr = probs_p.tile([128, 1024], BF16, tag="pr")
                    for i in (0, 1):
                        kt = 2 * p + i
                        koff = b * T + kt * 128
                        nc.tensor.matmul(sup[:, i * 512:(i + 1) * 512],
                                         kt_sb[hsl, koff:koff + 128],
                                         qt_sb[hsl, q0:q0 + QB],
                                         start=True, stop=True,
                                         skip_group_check=True)
                        if i == 0:
                            yield
                    nc.scalar.activation(out=pr, in_=sup, func=AF.Exp, scale=scale)
                    for i in (0, 1):
                        kt = 2 * p + i
                        koff = b * T + kt * 128
                        d = kt - 4 * j
                        c0 = 128 * d if d > 0 else 0
                        if d >= 0:
                            msl = slice(i * 512 + 128 * d, i * 512 + 128 * (d + 1))
                            nc.vector.tensor_mul(pr[:, msl], pr[:, msl], tri_sb)
                        pend.append((pa, q0, koff,
                                     pr[:, i * 512 + c0:(i + 1) * 512], c0,
                                     kt == 0, kt == nkt - 1))
                        if len(pend) > 4:
                            _flush(pend.pop(0))
                    if fill_cell:
                        # independent always-ready matmuls so the exp-paced
                        # micro-idles don't let HAM re-throttle the PE clock
                        for _ in range(3):
                            nc.tensor.matmul(fill_cell[0][:, 0:128], ones_sb,
                                             ones_sb, start=True, stop=True,
                                             skip_group_check=True)
                    yield
            while pend:
                _flush(pend.pop(0))
                yield

        def run_gens(gens, max_steps=None):
            steps = 0
            while gens:
                for g in list(gens):
                    try:
                        next(g)
                    except StopIteration:
                        gens.remove(g)
                        continue
                    steps += 1
                    if max_steps is not None and steps >= max_steps:
                        return
            return

        # phase 1 for batch 0, then batch-0 attention interleaved with
        # batch-1 phase 1 (exp on ACT overlaps QKV on PE), then batch-1 attn
        for j in range(4):
            stats_qkv_chunk(j)
        derive_batch(0)
        for j in range(4):
            fold_chunk(j)
        gens = [attn_group(0, 0), attn_group(0, 1)]
        for j in range(4, NCH):
            stats_qkv_chunk(j)
            run_gens(gens, max_steps=8)
        derive_batch(1)
        for j in range(4, NCH):
            fold_chunk(j)
            run_gens(gens, max_steps=4)
        # phase 1 done -> mm bank is free; use it as the HAM filler target
        warm2 = ps_mm.tile([128, 512], FP32, tag="mm")
        fill_cell.append(warm2)
        run_gens(gens)
        gens = [attn_group(1, 0), attn_group(1, 1)]
        run_gens(gens)
    nc.compile()
    return nc


def build_l2(use_beta2=True):
    """Feature-major L2: proj -> x2T directly, LN2 stats via 1/C-folded
    ones-matmuls overlapped with proj, FFN1/FFN2 with streamed weights, no PE
    transposes. Output stays transposed [C, ROWS]; host untransposes."""
    nc = bacc.Bacc("TRN2", target_bir_lowering=False, debug=False, num_devices=NCORES)
    at_d = nc.dram_tensor("at", [C, ROWS], BF16, kind="ExternalInput").ap()
    wp_d = nc.dram_tensor("wp", [C, C], BF16, kind="ExternalInput").ap()
    xrt_d = nc.dram_tensor("xrt", [C, ROWS], FP32, kind="ExternalInput").ap()
    w1_d = nc.dram_tensor("w1q", [128, HT, CT * 128], BF16, kind="ExternalInput").ap()
    w2_d = nc.dram_tensor("w2q", [128, CT, HT * 128], BF16, kind="ExternalInput").ap()
    b1_d = nc.dram_tensor("b1", [HID, 1], FP32, kind="ExternalInput").ap()
    b2_d = nc.dram_tensor("b2q", [128, CT], FP32, kind="ExternalInput").ap()
    bt2_d = nc.dram_tensor("beta2q", [128, CT], FP32, kind="ExternalInput").ap()
    out_d = nc.dram_tensor("out_t", [C, ROWS], FP32, kind="ExternalOutput").ap()

    with tile.TileContext(nc) as tc, contextlib.ExitStack() as ctx:
        consts = ctx.enter_context(tc.tile_pool(name="consts", bufs=1))
        persist = ctx.enter_context(tc.tile_pool(name="persist", bufs=1))
        wstream = ctx.enter_context(tc.tile_pool(name="wstream", bufs=3))
        w2stream = ctx.enter_context(tc.tile_pool(name="w2stream", bufs=2))
        tmp = ctx.enter_context(tc.tile_pool(name="tmp", bufs=3))
        small = ctx.enter_context(tc.tile_pool(name="small", bufs=4))
        ps_pj = ctx.enter_context(tc.tile_pool(name="ps_pj", bufs=1, space="PSUM"))
        ps_st = ctx.enter_context(tc.tile_pool(name="ps_st", bufs=1, space="PSUM"))
        ps_mm = ctx.enter_context(tc.tile_pool(name="ps_mm", bufs=2, space="PSUM"))

        ones_w = consts.tile([128, 128], BF16)
        nc.vector.memset(ones_w, 1.0 / C)
        eps_sb = consts.tile([128, 1], FP32)
        nc.vector.memset(eps_sb, EPS)
        # warm the PE (HAM un-throttles after ~3.4us of sustained activity)
        warm_ps = ps_mm.tile([128, 512], FP32, tag="mm")
        for _ in range(16):
            nc.tensor.matmul(warm_ps[:, 0:128], ones_w, ones_w[:, 0:128],
                             start=True, stop=True)

        b1_sb = consts.tile([128, HT], FP32)
        nc.sync.dma_start(out=b1_sb, in_=b1_d.rearrange("(a p) one -> p (a one)", p=128))
        b2_sb = consts.tile([128, CT], FP32)
        nc.sync.dma_start(out=b2_sb, in_=b2_d)
        bt2_sb = consts.tile([128, CT], FP32)
        nc.sync.dma_start(out=bt2_sb, in_=bt2_d)

        # packed input tiles, few big DMAs (each spreads over 16 SDMA engines)
        at2 = persist.tile([128, CT, ROWS], BF16, tag="at2")
        wp2 = persist.tile([128, CT, C], BF16, tag="wp2")
        xrt2 = persist.tile([128, CT, ROWS], FP32, tag="xrt2")
        at_r = at_d.rearrange("(a p) m -> p a m", p=128)
        wp_r = wp_d.rearrange("(a p) m -> p a m", p=128)
        xrt_r = xrt_d.rearrange("(a p) m -> p a m", p=128)
        for h in range(2):
            ds = slice(h * 4, (h + 1) * 4)
            nc.sync.dma_start(out=at2[:, ds, :], in_=at_r[:, ds, :])
            nc.sync.dma_start(out=wp2[:, ds, :], in_=wp_r[:, ds, :])
        nc.sync.dma_start(out=xrt2, in_=xrt_r)
        at_sb = [at2[:, d, :] for d in range(CT)]
        wp_sb = [wp2[:, d, :] for d in range(CT)]
        xrt_sb = [xrt2[:, n, :] for n in range(CT)]

        # ---- proj (x2T[n] = sum_d wp[d,n].T @ atT[d] + xrT[n]) with LN2
        # stats accumulating as each n-tile drains ----
        x2t_sb = []
        for n in range(CT):
            x2t = persist.tile([128, ROWS], FP32, tag=f"x2t{n}")
            x2t_sb.append(x2t)
        ps_sum = ps_st.tile([128, 512], FP32, tag="sum")
        ps_sq = ps_st.tile([128, 512], FP32, tag="sq")
        for half in range(2):
            pps = []
            for i in range(4):
                pj = ps_pj.tile([128, 512], FP32, tag=f"pj{i}")
                pps.append(pj)
            for d in range(CT):
                for i in range(4):
                    n = half * 4 + i
                    nc.tensor.matmul(pps[i], wp_sb[d][:, n * 128:(n + 1) * 128],
                                     at_sb[d], start=(d == 0), stop=(d == CT - 1),
                                     skip_group_check=True)
            for i in range(4):
                n = half * 4 + i
                nc.vector.tensor_add(x2t_sb[n], pps[i], xrt_sb[n])
                xb = tmp.tile([128, 512], BF16, tag="xb")
                nc.vector.tensor_copy(xb, x2t_sb[n])
                sq = tmp.tile([128, 512], BF16, tag="sqt")
                nc.vector.tensor_mul(sq, xb, xb)
                nc.tensor.matmul(ps_sum, ones_w, xb, start=(n == 0),
                                 stop=(n == CT - 1), skip_group_check=True)
                nc.tensor.matmul(ps_sq, ones_w, sq, start=(n == 0),
                                 stop=(n == CT - 1), skip_group_check=True)

        # ---- LN2 derive: rstd_b / -mu*rstd_b, broadcast across partitions.
        # Keep-warm spins are data-anchored on mid-derive values so the
        # scheduler must place them inside the PE bubble (HAM stays 8/8). ----
        mu2 = small.tile([128, 512], FP32, tag="mu2")
        nc.scalar.activation(out=mu2, in_=ps_sum, func=AF.Square)
        mu_sb = small.tile([128, 512], FP32, tag="mu")
        nc.vector.tensor_copy(mu_sb, ps_sum)
        mub16 = small.tile([128, 512], BF16, tag="mub16")
        nc.vector.tensor_copy(mub16, ps_sum)
        var = small.tile([128, 512], FP32, tag="var")
        nc.vector.tensor_sub(var, ps_sq, mu2)
        for _ in range(2):
            nc.tensor.matmul(warm_ps[:, 0:128], ones_w, mub16[:, 0:128],
                             start=True, stop=True, skip_group_check=True)
        nc.scalar.activation(out=var, in_=var, func=AF.Ln, bias=eps_sb, scale=1.0)
        rstd_b = small.tile([128, 512], FP32, tag="rstd")
        nc.scalar.activation(out=rstd_b, in_=var, func=AF.Exp, scale=-0.5)
        nmr_b = small.tile([128, 512], FP32, tag="nmr")
        nc.vector.scalar_tensor_tensor(nmr_b, mu_sb, -1.0, rstd_b,
                                       op0=mybir.AluOpType.mult,
                                       op1=mybir.AluOpType.mult)
        nmr16 = small.tile([128, 512], BF16, tag="nmr16")
        nc.vector.tensor_copy(nmr16, nmr_b)
        for _ in range(2):
            nc.tensor.matmul(warm_ps[:, 0:128], ones_w, nmr16[:, 0:128],
                             start=True, stop=True, skip_group_check=True)

        # ---- h2T = x2T*rstd + nmr (+beta2), bf16 ----
        h2t_sb = []
        for n in range(CT):
            h2t = persist.tile([128, ROWS], BF16, tag=f"h2t{n}")
            t = tmp.tile([128, 512], FP32, tag="h2f")
            nc.vector.tensor_mul(t, x2t_sb[n], rstd_b)
            nc.vector.tensor_add(h2t, t, nmr_b)
            if use_beta2:
                nc.vector.tensor_scalar_add(h2t, h2t, bt2_sb[:, n:n + 1])
            h2t_sb.append(h2t)

        # ---- FFN1: h1T[ht] = relu(W1g.T @ h2T + b1); w1 streamed in 1MB
        # groups of 4 ht-tiles ----
        h1t_sb = []
        for hg in range(HT // 4):
            w1g = wstream.tile([128, 4, CT, 128], BF16, tag="w1g")
            nc.sync.dma_start(
                out=w1g,
                in_=w1_d[:, hg * 4:(hg + 1) * 4, :].rearrange(
                    "p g (a n) -> p g a n", a=CT))
            for k in range(4):
                ht = hg * 4 + k
                ph = ps_mm.tile([128, 512], FP32, tag="mm")
                for ci in range(CT):
                    nc.tensor.matmul(ph, w1g[:, k, ci, :], h2t_sb[ci],
                                     start=(ci == 0), stop=(ci == CT - 1))
                h1 = persist.tile([128, ROWS], BF16, tag=f"h1t{ht}")
                nc.scalar.activation(out=h1, in_=ph, func=AF.Relu,
                                     bias=b1_sb[:, ht:ht + 1], scale=1.0)
                h1t_sb.append(h1)

        # ---- FFN2 (outT[n] = sum_ht w2[ht,n].T @ h1T[ht] + x2T[n] + b2) ----
        for n in range(CT):
            w2n = w2stream.tile([128, HT, 128], BF16, tag="w2n")
            nc.sync.dma_start(out=w2n,
                              in_=w2_d[:, n, :].rearrange("p (a m) -> p a m", a=HT))
            po = ps_mm.tile([128, 512], FP32, tag="mm")
            for ht in range(HT):
                nc.tensor.matmul(po, w2n[:, ht, :], h1t_sb[ht],
                                 start=(ht == 0), stop=(ht == HT - 1))
            ot = tmp.tile([128, 512], FP32, tag="ot")
            nc.vector.tensor_add(ot, po, x2t_sb[n])
            nc.vector.tensor_scalar_add(ot, ot, b2_sb[:, n:n + 1])
            nc.sync.dma_start(out=out_d[n * 128:(n + 1) * 128, :], in_=ot)
    nc.compile()
    return nc


# ---------------- host glue ----------------

def prep_l1_inputs(inputs):
    x = np.asarray(inputs["x"], np.float32).reshape(TOK, C)
    g1 = np.asarray(inputs["g1"], np.float32)
    beta1 = np.asarray(inputs["beta1"], np.float32)
    xt = np.ascontiguousarray(x.T).astype(bf16)
    wq = (g1[:, None] * np.asarray(inputs["Wq"], np.float32)).astype(bf16)
    wk = (g1[:, None] * np.asarray(inputs["Wk"], np.float32)).astype(bf16)
    wv = (g1[:, None] * np.asarray(inputs["Wv"], np.float32)).astype(bf16)
    tri = np.triu(np.ones((128, 128), np.float32)).astype(bf16)
    in_maps = []
    for c in range(NCORES):
        csl = slice(c * 128, (c + 1) * 128)
        nws = np.stack([-wq[:, csl].astype(np.float32).sum(0),
                        -wk[:, csl].astype(np.float32).sum(0),
                        -wv[:, csl].astype(np.float32).sum(0)], axis=1)
        wb = np.stack([wq[:, csl].astype(np.float32).T @ beta1,
                       wk[:, csl].astype(np.float32).T @ beta1,
                       wv[:, csl].astype(np.float32).T @ beta1], axis=1)
        in_maps.append({
            "xt": xt,
            "wq": np.ascontiguousarray(wq[:, csl]),
            "wk": np.ascontiguousarray(wk[:, csl]),
            "wv": np.ascontiguousarray(wv[:, csl]),
            "nws": np.ascontiguousarray(nws.astype(np.float32)),
            "wb": np.ascontiguousarray(wb.astype(np.float32)),
            "tri": tri,
        })
    return in_maps


def prep_l2_inputs(inputs, attn_t):
    attn_t = np.ascontiguousarray(np.asarray(attn_t, bf16))
    x = np.asarray(inputs["x"], np.float32).reshape(TOK, C)
    g2 = np.asarray(inputs["g2"], np.float32)
    wp = np.asarray(inputs["Wp"], np.float32).astype(bf16)
    w1 = (g2[:, None] * np.asarray(inputs["W1"], np.float32)).astype(bf16)
    w1q = np.ascontiguousarray(
        w1.reshape(CT, 128, HT, 128).transpose(1, 2, 0, 3).reshape(128, HT, CT * 128))
    w2 = np.asarray(inputs["W2"], np.float32).astype(bf16)
    # w2q[p, n, ht*128+c] = W2[ht*128+p, n*128+c]
    w2q = np.ascontiguousarray(
        w2.reshape(HT, 128, CT, 128).transpose(1, 2, 0, 3).reshape(128, CT, HT * 128))
    b1 = np.ascontiguousarray(np.asarray(inputs["b1"], np.float32).reshape(HID, 1))
    xt = np.ascontiguousarray(
        (x + np.asarray(inputs["bp"], np.float32)[None, :]).T)
    beta2q = np.ascontiguousarray(
        np.asarray(inputs["beta2"], np.float32).reshape(CT, 128).T)
    b2q = np.ascontiguousarray(
        np.asarray(inputs["b2"], np.float32).reshape(CT, 128).T)
    in_maps = []
    for c in range(NCORES):
        rsl = slice(c * ROWS, (c + 1) * ROWS)
        in_maps.append({
            "at": np.ascontiguousarray(attn_t[:, rsl]),
            "wp": wp,
            "xrt": np.ascontiguousarray(xt[:, rsl]),
            "w1q": w1q,
            "w2q": w2q,
            "b1": b1,
            "beta2q": beta2q,
            "b2q": b2q,
        })
    return in_maps


_CACHE = {}


def _get_programs(use_beta, use_beta2):
    key = ("progs", bool(use_beta), bool(use_beta2))
    if key not in _CACHE:
        nc1 = build_l1(use_beta=use_beta)
        nc2 = build_l2(use_beta2=use_beta2)
        _CACHE[key] = (nc1, nc2)
    return _CACHE[key]


def kernel(**inputs):
    from concourse.bass_utils import run_bass_kernel_spmd

    inputs = {k: np.asarray(v) for k, v in inputs.items()}
    use_beta = bool(np.any(np.asarray(inputs["beta1"], np.float32) != 0.0))
    use_beta2 = bool(np.any(np.asarray(inputs["beta2"], np.float32) != 0.0))
    nc1, nc2 = _get_programs(use_beta, use_beta2)
    core_ids = list(range(NCORES))

    r1 = run_bass_kernel_spmd(nc1, prep_l1_inputs(inputs), core_ids)
    # unshard + normalize: rows 0-63 h0 attn, 64 h0 denom, 65-128 h1, 129 h1 denom
    parts = []
    for c in range(NCORES):
        o = np.asarray(r1.results[c]["attn_out"], np.float32)
        parts.append(o[0:64] / o[64:65])
        parts.append(o[65:129] / o[129:130])
    attn_t = np.concatenate(parts, axis=0)

    r2 = run_bass_kernel_spmd(nc2, prep_l2_inputs(inputs, attn_t), core_ids)
    out = np.concatenate(
        [np.asarray(r2.results[c]["out_t"]) for c in range(NCORES)], axis=1)
    return np.ascontiguousarray(out.T.reshape(B, T, C).astype(np.float32))



# revision 26
# speedup vs baseline: 1.2472x; 1.2472x over previous
"""Pre-LN transformer block (B=2,T=2048,C=1024,H=16) on 8 TRN2 NeuronCores.

Two SPMD launches:
  L1: tensor-parallel over heads (2 heads/core) - LN1 stats on-chip
      (replicated, folded algebraically into the QKV matmuls), causal
      attention with on-chip softmax, normalized attn^T output per core.
  L2: parallel over token rows (512 rows/core) - output projection +
      residual, LN2, FFN (relu) + residual.
Host work between launches is just resharding: slicing/concat and dtype
casts. All matmuls run in bf16 with fp32 PSUM accumulation; residuals
are carried in fp32.
"""
"""Transformer block on 8 TRN2 cores: L1 = head-parallel attention, L2 = row-parallel proj+FFN."""
import contextlib
import numpy as np
import ml_dtypes

import concourse.bass as bass
import concourse.mybir as mybir
import concourse.tile as tile
from concourse import bacc
from concourse.masks import make_identity

bf16 = ml_dtypes.bfloat16
FP32 = mybir.dt.float32
BF16 = mybir.dt.bfloat16
AF = mybir.ActivationFunctionType

B, T, C, H = 2, 2048, 1024, 16
HS = C // H          # 64
NCORES = 8
HPC = H // NCORES    # 2 heads per core
TOK = B * T          # 4096
EPS = 1e-5
CT = C // 128        # 8 c-tiles
NCH = TOK // 512     # 8 512-col chunks of token axis
QB = 512             # query block
ROWS = TOK // NCORES # 512 rows per core in L2
HID = 4 * C          # 4096
HT = HID // 128      # 32 hidden tiles
MT = ROWS // 128     # 4 token tiles in L2


def build_l1(use_beta=True):
    """L1: head-parallel attention (2 heads/core).

    Phase 1 (per 512-token chunk, interleaved so PE stays dense while xT
    streams in): LN1 stats via 1/C-folded ones-matmuls (born broadcast),
    QKV on raw xT with the LN fold applied at drain, V' transpose tiles.
    Phase 2: causal attention with k-tile-PAIRED exp (one ACT instruction
    per [128,1024] PSUM supertile spanning 2 banks) and 65-wide attn-V
    (ones column = softmax denominator). Output is UNNORMALIZED attn plus
    denominators [130, TOK] fp32; the host divides (free) before L2.
    Batch-0 attention is emitted interleaved with batch-1's phase 1 so the
    ACT-bound exp overlaps the PE-bound QKV work.
    """
    nc = bacc.Bacc("TRN2", target_bir_lowering=False, debug=False, num_devices=NCORES)
    xt_d = nc.dram_tensor("xt", [C, TOK], BF16, kind="ExternalInput").ap()
    wq_d = nc.dram_tensor("wq", [C, 128], BF16, kind="ExternalInput").ap()
    wk_d = nc.dram_tensor("wk", [C, 128], BF16, kind="ExternalInput").ap()
    wv_d = nc.dram_tensor("wv", [C, 128], BF16, kind="ExternalInput").ap()
    # negated column sums of wq/wk/wv and W.T @ beta1, all [128,1] fp32
    nws_d = nc.dram_tensor("nws", [128, 3], FP32, kind="ExternalInput").ap()
    wb_d = nc.dram_tensor("wb", [128, 3], FP32, kind="ExternalInput").ap()
    tri_d = nc.dram_tensor("tri", [128, 128], BF16, kind="ExternalInput").ap()
    # rows 0-63: h0 attn (unnormalized), 64: h0 denom, 65-128: h1, 129: h1 denom
    out_d = nc.dram_tensor("attn_out", [130, TOK], FP32, kind="ExternalOutput").ap()

    with tile.TileContext(nc) as tc, contextlib.ExitStack() as ctx:
        consts = ctx.enter_context(tc.tile_pool(name="consts", bufs=1))
        hpool = ctx.enter_context(tc.tile_pool(name="hT", bufs=1))
        stats = ctx.enter_context(tc.tile_pool(name="stats", bufs=1))
        tmp = ctx.enter_context(tc.tile_pool(name="tmp", bufs=3))
        qkv = ctx.enter_context(tc.tile_pool(name="qkv", bufs=1))
        probs_p = ctx.enter_context(tc.tile_pool(name="probs", bufs=6))
        attn_sb_p = ctx.enter_context(tc.tile_pool(name="attn_sb", bufs=4))
        # PSUM budget (8 banks): sup0 + sup1 ([128,1024] = 2 banks each),
        # pa0 + pa1 (1 each), mm (1), tr (1)
        ps_sup = ctx.enter_context(tc.tile_pool(name="ps_sup", bufs=1, space="PSUM"))
        ps_acc = ctx.enter_context(tc.tile_pool(name="ps_acc", bufs=1, space="PSUM"))
        ps_mm = ctx.enter_context(tc.tile_pool(name="ps_mm", bufs=1, space="PSUM"))
        ps_tr = ctx.enter_context(tc.tile_pool(name="ps_tr", bufs=1, space="PSUM"))

        # ---- xT loads. Everything rides the sync HW-DGE ring so transfers
        # complete in issue order (no SW-DGE round-robin stealing bandwidth
        # from the critical early chunks): chunk 0 per-ci first (first tile
        # arrives in a few us and each ci arrival re-arms HAM via the stats
        # matmuls), then consts, then the later chunks.
        xbig = hpool.tile([128, CT, TOK], BF16, tag="xbig")
        xt_r = xt_d.rearrange("(a p) m -> p a m", p=128)
        xts = [xbig[:, ci, :] for ci in range(CT)]
        for ci in range(CT):
            nc.sync.dma_start(out=xbig[:, ci, 0:512], in_=xt_r[:, ci, 0:512])

        # ---- constants ----
        ones_sb = consts.tile([128, 128], BF16)
        nc.vector.memset(ones_sb, 1.0 / C)
        eps_sb = consts.tile([128, 1], FP32)
        nc.vector.memset(eps_sb, EPS)
        ident = consts.tile([128, 128], BF16)
        make_identity(nc, ident)

        wq_sb = consts.tile([128, CT, 128], BF16)
        nc.sync.dma_start(out=wq_sb, in_=wq_d.rearrange("(a p) m -> p a m", p=128))
        wk_sb = consts.tile([128, CT, 128], BF16)
        nc.sync.dma_start(out=wk_sb, in_=wk_d.rearrange("(a p) m -> p a m", p=128))
        wv_sb = consts.tile([128, CT, 128], BF16)
        nc.sync.dma_start(out=wv_sb, in_=wv_d.rearrange("(a p) m -> p a m", p=128))
        nws_sb = consts.tile([128, 3], FP32)
        nc.sync.dma_start(out=nws_sb, in_=nws_d)
        wb_sb = consts.tile([128, 3], FP32)
        nc.sync.dma_start(out=wb_sb, in_=wb_d)
        tri_sb = consts.tile([128, 128], BF16)
        nc.sync.dma_start(out=tri_sb, in_=tri_d)

        for ci in range(CT):
            nc.sync.dma_start(out=xbig[:, ci, 512:2048], in_=xt_r[:, ci, 512:2048])
        for ci in range(CT):
            nc.sync.dma_start(out=xbig[:, ci, 2048:4096],
                              in_=xt_r[:, ci, 2048:4096])

        # PE warm-up spin: flips HAM to 8/8 while inputs stream, then a
        # data-anchored ladder (each spin waits its ci's chunk-0 DMA) keeps
        # HAM warm across the whole arrival window at zero real cost
        warm_ps = ps_mm.tile([128, 512], FP32, tag="mm")
        for _ in range(10):
            nc.tensor.matmul(warm_ps[:, 0:128], ones_sb, ones_sb[:, 0:128],
                             start=True, stop=True)
        for ci in range(CT):
            nc.tensor.matmul(warm_ps[:, 0:128], ones_sb, xts[ci][:, 0:128],
                             start=True, stop=True, skip_group_check=True)
            nc.tensor.matmul(warm_ps[:, 0:128], ones_sb, ones_sb[:, 0:128],
                             start=True, stop=True, skip_group_check=True)

        rstd_b = stats.tile([128, TOK], BF16, tag="rstd_b")
        murstd_b = stats.tile([128, TOK], BF16, tag="murstd_b")
        qt_sb = qkv.tile([128, TOK], BF16, tag="qt")
        kt_sb = qkv.tile([128, TOK], BF16, tag="kt")
        vt_sb = qkv.tile([128, TOK], BF16, tag="vt")
        vts = []
        for tt in range(TOK // 128):
            vt = qkv.tile([128, 130], BF16, tag=f"v{tt}")
            vts.append(vt)
        mu_bat, var_bat = {}, {}
        for h in range(2):
            mb = stats.tile([128, 2048], FP32, tag=f"muB{h}")
            vb = stats.tile([128, 2048], FP32, tag=f"varB{h}")
            mu_bat[h], var_bat[h] = mb, vb

        def stats_qkv_chunk(j):
            """stats matmuls + derive-to-var (no ACT Ln yet) + raw QKV drains."""
            sl = slice(j * 512, (j + 1) * 512)
            bsl = slice((j % 4) * 512, (j % 4 + 1) * 512)
            mu, var = mu_bat[j // 4][:, bsl], var_bat[j // 4][:, bsl]
            st = ps_sup.tile([128, 1024], FP32, tag=f"sup{j % 2}")
            for ci in range(CT):
                sq = tmp.tile([128, 512], BF16, tag="sq")
                nc.vector.tensor_mul(sq, xts[ci][:, sl], xts[ci][:, sl])
                nc.tensor.matmul(st[:, 0:512], ones_sb, xts[ci][:, sl],
                                 start=(ci == 0), stop=(ci == CT - 1),
                                 skip_group_check=True)
                nc.tensor.matmul(st[:, 512:1024], ones_sb, sq,
                                 start=(ci == 0), stop=(ci == CT - 1),
                                 skip_group_check=True)
            nc.vector.tensor_copy(mu, st[:, 0:512])
            mu2 = tmp.tile([128, 512], FP32, tag="mu2")
            nc.vector.tensor_mul(mu2, mu, mu)
            nc.vector.tensor_sub(var, st[:, 512:1024], mu2)
            # QKV raw drains (fold applied after the batched Ln/Exp pass)
            for wsb, idx, tsb in ((wq_sb, 0, qt_sb), (wk_sb, 1, kt_sb), (wv_sb, 2, vt_sb)):
                pp = ps_mm.tile([128, 512], FP32, tag="mm")
                for ci in range(CT):
                    nc.tensor.matmul(pp, wsb[:, ci, :], xts[ci][:, sl],
                                     start=(ci == 0), stop=(ci == CT - 1))
                nc.scalar.copy(out=tsb[:, sl], in_=pp)

        def derive_batch(h):
            """rstd for 4 chunks in ONE wide Ln and ONE wide Exp instruction —
            single instructions can't be interleaved by the scheduler, so the
            ACT table set switches exactly twice per batch."""
            hsl = slice(h * 2048, (h + 1) * 2048)
            nc.scalar.activation(out=var_bat[h], in_=var_bat[h], func=AF.Ln,
                                 bias=eps_sb, scale=1.0)
            nc.scalar.activation(out=rstd_b[:, hsl], in_=var_bat[h], func=AF.Exp,
                                 scale=-0.5)
            nc.vector.tensor_mul(murstd_b[:, hsl], mu_bat[h], rstd_b[:, hsl])
            # keep-warm spins anchored on the just-produced stats so the
            # scheduler can't hoist them out of the derive bubble (fresh mm
            # tile so the pool serializes them against the last QKV drain)
            wsp = ps_mm.tile([128, 512], FP32, tag="mm")
            for _ in range(2):
                nc.tensor.matmul(wsp[:, 0:128], ones_sb,
                                 murstd_b[:, h * 2048:h * 2048 + 128],
                                 start=True, stop=True, skip_group_check=True)

        def fold_chunk(j):
            """apply the LN fold to the raw QKV drains, then build V' tiles."""
            sl = slice(j * 512, (j + 1) * 512)
            for idx, tsb in ((0, qt_sb), (1, kt_sb), (2, vt_sb)):
                nc.vector.tensor_mul(tsb[:, sl], tsb[:, sl], rstd_b[:, sl])
                nc.vector.scalar_tensor_tensor(
                    tsb[:, sl], murstd_b[:, sl], nws_sb[:, idx:idx + 1], tsb[:, sl],
                    op0=mybir.AluOpType.mult, op1=mybir.AluOpType.add)
                if use_beta:
                    nc.vector.tensor_scalar_add(tsb[:, sl], tsb[:, sl],
                                                wb_sb[:, idx:idx + 1])
            # V' tiles [128,130] = [v_h0 | ones | v_h1 | ones]
            for tt in range(j * 4, j * 4 + 4):
                vt = vts[tt]
                ptv = ps_tr.tile([128, 128], BF16, tag="tr")
                nc.tensor.transpose(ptv, vt_sb[:, tt * 128:(tt + 1) * 128], ident)
                nc.vector.tensor_copy(
                    vt.rearrange("p (g c) -> p g c", g=2)[:, :, 0:64],
                    ptv.rearrange("p (g c) -> p g c", g=2))
                nc.vector.memset(vt[:, 64:65], 1.0)
                nc.vector.memset(vt[:, 129:130], 1.0)

        # ---- attention generators (paired k-tiles -> one exp per 2 banks) ----
        scale = C ** -0.5
        # HAM keep-warm filler target; set once phase 1 stops using the mm bank
        fill_cell = []

        def attn_group(b, hl):
            hsl = slice(hl * 64, (hl + 1) * 64)
            vcol = slice(hl * 65, hl * 65 + 65)
            pend = []

            def _flush(item):
                pa_, q0_, koff_, pr_ap_, c0_, st_, sp_ = item
                nc.tensor.matmul(pa_[:, c0_:] if c0_ else pa_,
                                 vts[koff_ // 128][:, vcol], pr_ap_,
                                 start=st_, stop=sp_, skip_group_check=True)
                if sp_:
                    asb = attn_sb_p.tile([65, 512], FP32, tag="asb")
                    nc.vector.tensor_copy(asb, pa_)
                    nc.sync.dma_start(
                        out=out_d[hl * 65:hl * 65 + 65, q0_:q0_ + QB], in_=asb)

            for j in range(T // QB):
                q0 = b * T + j * QB
                pa = ps_acc.tile([65, 512], FP32, tag=f"pa{hl}")
                nkt = 4 * (j + 1)
                for p in range(nkt // 2):
                    sup = ps_sup.tile([128, 1024], FP32, tag=f"sup{hl}")
                    pr = probs_p.tile([128, 1024], BF16, tag="pr")
                    for i in (0, 1):
                        kt = 2 * p + i
                        koff = b * T + kt * 128
                        nc.tensor.matmul(sup[:, i * 512:(i + 1) * 512],
                                         kt_sb[hsl, koff:koff + 128],
                                         qt_sb[hsl, q0:q0 + QB],
                                         start=True, stop=True,
                                         skip_group_check=True)
                        if i == 0:
                            yield
                    nc.scalar.activation(out=pr, in_=sup, func=AF.Exp, scale=scale)
                    for i in (0, 1):
                        kt = 2 * p + i
                        koff = b * T + kt * 128
                        d = kt - 4 * j
                        c0 = 128 * d if d > 0 else 0
                        if d >= 0:
                            msl = slice(i * 512 + 128 * d, i * 512 + 128 * (d + 1))
                            nc.vector.tensor_mul(pr[:, msl], pr[:, msl], tri_sb)
                        pend.append((pa, q0, koff,
                                     pr[:, i * 512 + c0:(i + 1) * 512], c0,
                                     kt == 0, kt == nkt - 1))
                        if len(pend) > 4:
                            _flush(pend.pop(0))
                    if fill_cell:
                        # independent always-ready matmuls so the exp-paced
                        # micro-idles don't let HAM re-throttle the PE clock
                        for _ in range(3):
                            nc.tensor.matmul(fill_cell[0][:, 0:128], ones_sb,
                                             ones_sb, start=True, stop=True,
                                             skip_group_check=True)
                    yield
            while pend:
                _flush(pend.pop(0))
                yield

        def run_gens(gens, max_steps=None):
            steps = 0
            while gens:
                for g in list(gens):
                    try:
                        next(g)
                    except StopIteration:
                        gens.remove(g)
                        continue
                    steps += 1
                    if max_steps is not None and steps >= max_steps:
                        return
            return

        # phase 1 for batch 0, then batch-0 attention interleaved with
        # batch-1 phase 1 (exp on ACT overlaps QKV on PE), then batch-1 attn
        for j in range(4):
            stats_qkv_chunk(j)
        derive_batch(0)
        for j in range(4):
            fold_chunk(j)
        gens = [attn_group(0, 0), attn_group(0, 1)]
        for j in range(4, NCH):
            stats_qkv_chunk(j)
            run_gens(gens, max_steps=8)
        derive_batch(1)
        for j in range(4, NCH):
            fold_chunk(j)
            run_gens(gens, max_steps=4)
        # phase 1 done -> mm bank is free; use it as the HAM filler target
        warm2 = ps_mm.tile([128, 512], FP32, tag="mm")
        fill_cell.append(warm2)
        run_gens(gens)
        gens = [attn_group(1, 0), attn_group(1, 1)]
        run_gens(gens)
    nc.compile()
    return nc


def build_l2(use_beta2=True):
    """Feature-major L2: proj -> x2T directly, LN2 stats via 1/C-folded
    ones-matmuls overlapped with proj, FFN1/FFN2 with streamed weights, no PE
    transposes. Output stays transposed [C, ROWS]; host untransposes."""
    nc = bacc.Bacc("TRN2", target_bir_lowering=False, debug=False, num_devices=NCORES)
    at_d = nc.dram_tensor("at", [C, ROWS], BF16, kind="ExternalInput").ap()
    wp_d = nc.dram_tensor("wp", [C, C], BF16, kind="ExternalInput").ap()
    xrt_d = nc.dram_tensor("xrt", [C, ROWS], FP32, kind="ExternalInput").ap()
    w1_d = nc.dram_tensor("w1q", [128, HT, CT * 128], BF16, kind="ExternalInput").ap()
    w2_d = nc.dram_tensor("w2q", [128, CT, HT * 128], BF16, kind="ExternalInput").ap()
    b1_d = nc.dram_tensor("b1", [HID, 1], FP32, kind="ExternalInput").ap()
    b2_d = nc.dram_tensor("b2q", [128, CT], FP32, kind="ExternalInput").ap()
    bt2_d = nc.dram_tensor("beta2q", [128, CT], FP32, kind="ExternalInput").ap()
    out_d = nc.dram_tensor("out_t", [C, ROWS], FP32, kind="ExternalOutput").ap()

    with tile.TileContext(nc) as tc, contextlib.ExitStack() as ctx:
        consts = ctx.enter_context(tc.tile_pool(name="consts", bufs=1))
        persist = ctx.enter_context(tc.tile_pool(name="persist", bufs=1))
        wstream = ctx.enter_context(tc.tile_pool(name="wstream", bufs=3))
        w2stream = ctx.enter_context(tc.tile_pool(name="w2stream", bufs=2))
        tmp = ctx.enter_context(tc.tile_pool(name="tmp", bufs=3))
        small = ctx.enter_context(tc.tile_pool(name="small", bufs=4))
        ps_pj = ctx.enter_context(tc.tile_pool(name="ps_pj", bufs=1, space="PSUM"))
        ps_st = ctx.enter_context(tc.tile_pool(name="ps_st", bufs=1, space="PSUM"))
        ps_mm = ctx.enter_context(tc.tile_pool(name="ps_mm", bufs=2, space="PSUM"))

        ones_w = consts.tile([128, 128], BF16)
        nc.vector.memset(ones_w, 1.0 / C)
        eps_sb = consts.tile([128, 1], FP32)
        nc.vector.memset(eps_sb, EPS)
        # packed input tiles, few big DMAs (each spreads over 16 SDMA engines)
        at2 = persist.tile([128, CT, ROWS], BF16, tag="at2")
        wp2 = persist.tile([128, CT, C], BF16, tag="wp2")
        xrt2 = persist.tile([128, CT, ROWS], FP32, tag="xrt2")
        at_r = at_d.rearrange("(a p) m -> p a m", p=128)
        wp_r = wp_d.rearrange("(a p) m -> p a m", p=128)
        xrt_r = xrt_d.rearrange("(a p) m -> p a m", p=128)
        for h in range(2):
            ds = slice(h * 4, (h + 1) * 4)
            nc.sync.dma_start(out=at2[:, ds, :], in_=at_r[:, ds, :])
            nc.sync.dma_start(out=wp2[:, ds, :], in_=wp_r[:, ds, :])
        nc.sync.dma_start(out=xrt2, in_=xrt_r)
        at_sb = [at2[:, d, :] for d in range(CT)]
        wp_sb = [wp2[:, d, :] for d in range(CT)]
        xrt_sb = [xrt2[:, n, :] for n in range(CT)]

        b1_sb = consts.tile([128, HT], FP32)
        nc.sync.dma_start(out=b1_sb, in_=b1_d.rearrange("(a p) one -> p (a one)", p=128))
        b2_sb = consts.tile([128, CT], FP32)
        nc.sync.dma_start(out=b2_sb, in_=b2_d)
        bt2_sb = consts.tile([128, CT], FP32)
        nc.sync.dma_start(out=bt2_sb, in_=bt2_d)

        # warm the PE (HAM un-throttles after ~3.4us of sustained activity),
        # then a ladder anchored on the arriving at/wp tiles spans the rest
        # of the DMA wait
        warm_ps = ps_mm.tile([128, 512], FP32, tag="mm")
        for _ in range(10):
            nc.tensor.matmul(warm_ps[:, 0:128], ones_w, ones_w[:, 0:128],
                             start=True, stop=True)
        for d in range(4):
            nc.tensor.matmul(warm_ps[:, 0:128], ones_w, at2[:, d, 0:128],
                             start=True, stop=True, skip_group_check=True)
            nc.tensor.matmul(warm_ps[:, 0:128], ones_w, ones_w[:, 0:128],
                             start=True, stop=True, skip_group_check=True)
            nc.tensor.matmul(warm_ps[:, 0:128], ones_w, wp2[:, d, 0:128],
                             start=True, stop=True, skip_group_check=True)
            nc.tensor.matmul(warm_ps[:, 0:128], ones_w, ones_w[:, 0:128],
                             start=True, stop=True, skip_group_check=True)

        # ---- proj (x2T[n] = sum_d wp[d,n].T @ atT[d] + xrT[n]) with LN2
        # stats accumulating as each n-tile drains ----
        x2t_sb = []
        for n in range(CT):
            x2t = persist.tile([128, ROWS], FP32, tag=f"x2t{n}")
            x2t_sb.append(x2t)
        ps_sum = ps_st.tile([128, 512], FP32, tag="sum")
        ps_sq = ps_st.tile([128, 512], FP32, tag="sq")
        for half in range(2):
            pps = []
            for i in range(4):
                pj = ps_pj.tile([128, 512], FP32, tag=f"pj{i}")
                pps.append(pj)
            for d in range(CT):
                for i in range(4):
                    n = half * 4 + i
                    nc.tensor.matmul(pps[i], wp_sb[d][:, n * 128:(n + 1) * 128],
                                     at_sb[d], start=(d == 0), stop=(d == CT - 1),
                                     skip_group_check=True)
            for i in range(4):
                n = half * 4 + i
                nc.vector.tensor_add(x2t_sb[n], pps[i], xrt_sb[n])
                xb = tmp.tile([128, 512], BF16, tag="xb")
                nc.vector.tensor_copy(xb, x2t_sb[n])
                sq = tmp.tile([128, 512], BF16, tag="sqt")
                nc.vector.tensor_mul(sq, xb, xb)
                nc.tensor.matmul(ps_sum, ones_w, xb, start=(n == 0),
                                 stop=(n == CT - 1), skip_group_check=True)
                nc.tensor.matmul(ps_sq, ones_w, sq, start=(n == 0),
                                 stop=(n == CT - 1), skip_group_check=True)

        # ---- LN2 derive: rstd_b / -mu*rstd_b, broadcast across partitions.
        # Keep-warm spins are data-anchored on mid-derive values so the
        # scheduler must place them inside the PE bubble (HAM stays 8/8). ----
        mu2 = small.tile([128, 512], FP32, tag="mu2")
        nc.scalar.activation(out=mu2, in_=ps_sum, func=AF.Square)
        mu_sb = small.tile([128, 512], FP32, tag="mu")
        nc.vector.tensor_copy(mu_sb, ps_sum)
        mub16 = small.tile([128, 512], BF16, tag="mub16")
        nc.vector.tensor_copy(mub16, ps_sum)
        var = small.tile([128, 512], FP32, tag="var")
        nc.vector.tensor_sub(var, ps_sq, mu2)
        for _ in range(2):
            nc.tensor.matmul(warm_ps[:, 0:128], ones_w, mub16[:, 0:128],
                             start=True, stop=True, skip_group_check=True)
        nc.scalar.activation(out=var, in_=var, func=AF.Ln, bias=eps_sb, scale=1.0)
        rstd_b = small.tile([128, 512], FP32, tag="rstd")
        nc.scalar.activation(out=rstd_b, in_=var, func=AF.Exp, scale=-0.5)
        nmr_b = small.tile([128, 512], FP32, tag="nmr")
        nc.vector.scalar_tensor_tensor(nmr_b, mu_sb, -1.0, rstd_b,
                                       op0=mybir.AluOpType.mult,
                                       op1=mybir.AluOpType.mult)
        nmr16 = small.tile([128, 512], BF16, tag="nmr16")
        nc.vector.tensor_copy(nmr16, nmr_b)
        for _ in range(2):
            nc.tensor.matmul(warm_ps[:, 0:128], ones_w, nmr16[:, 0:128],
                             start=True, stop=True, skip_group_check=True)

        # ---- h2T = x2T*rstd + nmr (+beta2), bf16 ----
        h2t_sb = []
        for n in range(CT):
            h2t = persist.tile([128, ROWS], BF16, tag=f"h2t{n}")
            t = tmp.tile([128, 512], FP32, tag="h2f")
            nc.vector.tensor_mul(t, x2t_sb[n], rstd_b)
            nc.vector.tensor_add(h2t, t, nmr_b)
            if use_beta2:
                nc.vector.tensor_scalar_add(h2t, h2t, bt2_sb[:, n:n + 1])
            h2t_sb.append(h2t)

        # ---- FFN1: h1T[ht] = relu(W1g.T @ h2T + b1); w1 streamed in 1MB
        # groups of 4 ht-tiles ----
        h1t_sb = []
        for hg in range(HT // 4):
            w1g = wstream.tile([128, 4, CT, 128], BF16, tag="w1g")
            nc.sync.dma_start(
                out=w1g,
                in_=w1_d[:, hg * 4:(hg + 1) * 4, :].rearrange(
                    "p g (a n) -> p g a n", a=CT))
            for k in range(4):
                ht = hg * 4 + k
                ph = ps_mm.tile([128, 512], FP32, tag="mm")
                for ci in range(CT):
                    nc.tensor.matmul(ph, w1g[:, k, ci, :], h2t_sb[ci],
                                     start=(ci == 0), stop=(ci == CT - 1))
                h1 = persist.tile([128, ROWS], BF16, tag=f"h1t{ht}")
                nc.scalar.activation(out=h1, in_=ph, func=AF.Relu,
                                     bias=b1_sb[:, ht:ht + 1], scale=1.0)
                h1t_sb.append(h1)

        # ---- FFN2 (outT[n] = sum_ht w2[ht,n].T @ h1T[ht] + x2T[n] + b2) ----
        for n in range(CT):
            w2n = w2stream.tile([128, HT, 128], BF16, tag="w2n")
            nc.sync.dma_start(out=w2n,
                              in_=w2_d[:, n, :].rearrange("p (a m) -> p a m", a=HT))
            po = ps_mm.tile([128, 512], FP32, tag="mm")
            for ht in range(HT):
                nc.tensor.matmul(po, w2n[:, ht, :], h1t_sb[ht],
                                 start=(ht == 0), stop=(ht == HT - 1))
            ot = tmp.tile([128, 512], FP32, tag="ot")
            nc.vector.tensor_add(ot, po, x2t_sb[n])
            nc.vector.tensor_scalar_add(ot, ot, b2_sb[:, n:n + 1])
            nc.sync.dma_start(out=out_d[n * 128:(n + 1) * 128, :], in_=ot)
    nc.compile()
    return nc


# ---------------- host glue ----------------

def prep_l1_inputs(inputs):
    x = np.asarray(inputs["x"], np.float32).reshape(TOK, C)
    g1 = np.asarray(inputs["g1"], np.float32)
    beta1 = np.asarray(inputs["beta1"], np.float32)
    xt = np.ascontiguousarray(x.T).astype(bf16)
    wq = (g1[:, None] * np.asarray(inputs["Wq"], np.float32)).astype(bf16)
    wk = (g1[:, None] * np.asarray(inputs["Wk"], np.float32)).astype(bf16)
    wv = (g1[:, None] * np.asarray(inputs["Wv"], np.float32)).astype(bf16)
    tri = np.triu(np.ones((128, 128), np.float32)).astype(bf16)
    in_maps = []
    for c in range(NCORES):
        csl = slice(c * 128, (c + 1) * 128)
        nws = np.stack([-wq[:, csl].astype(np.float32).sum(0),
                        -wk[:, csl].astype(np.float32).sum(0),
                        -wv[:, csl].astype(np.float32).sum(0)], axis=1)
        wb = np.stack([wq[:, csl].astype(np.float32).T @ beta1,
                       wk[:, csl].astype(np.float32).T @ beta1,
                       wv[:, csl].astype(np.float32).T @ beta1], axis=1)
        in_maps.append({
            "xt": xt,
            "wq": np.ascontiguousarray(wq[:, csl]),
            "wk": np.ascontiguousarray(wk[:, csl]),
            "wv": np.ascontiguousarray(wv[:, csl]),
            "nws": np.ascontiguousarray(nws.astype(np.float32)),
            "wb": np.ascontiguousarray(wb.astype(np.float32)),
            "tri": tri,
        })
    return in_maps


def prep_l2_inputs(inputs, attn_t):
    attn_t = np.ascontiguousarray(np.asarray(attn_t, bf16))
    x = np.asarray(inputs["x"], np.float32).reshape(TOK, C)
    g2 = np.asarray(inputs["g2"], np.float32)
    wp = np.asarray(inputs["Wp"], np.float32).astype(bf16)
    w1 = (g2[:, None] * np.asarray(inputs["W1"], np.float32)).astype(bf16)
    w1q = np.ascontiguousarray(
        w1.reshape(CT, 128, HT, 128).transpose(1, 2, 0, 3).reshape(128, HT, CT * 128))
    w2 = np.asarray(inputs["W2"], np.float32).astype(bf16)
    # w2q[p, n, ht*128+c] = W2[ht*128+p, n*128+c]
    w2q = np.ascontiguousarray(
        w2.reshape(HT, 128, CT, 128).transpose(1, 2, 0, 3).reshape(128, CT, HT * 128))
    b1 = np.ascontiguousarray(np.asarray(inputs["b1"], np.float32).reshape(HID, 1))
    xt = np.ascontiguousarray(
        (x + np.asarray(inputs["bp"], np.float32)[None, :]).T)
    beta2q = np.ascontiguousarray(
        np.asarray(inputs["beta2"], np.float32).reshape(CT, 128).T)
    b2q = np.ascontiguousarray(
        np.asarray(inputs["b2"], np.float32).reshape(CT, 128).T)
    in_maps = []
    for c in range(NCORES):
        rsl = slice(c * ROWS, (c + 1) * ROWS)
        in_maps.append({
            "at": np.ascontiguousarray(attn_t[:, rsl]),
            "wp": wp,
            "xrt": np.ascontiguousarray(xt[:, rsl]),
            "w1q": w1q,
            "w2q": w2q,
            "b1": b1,
            "beta2q": beta2q,
            "b2q": b2q,
        })
    return in_maps


_CACHE = {}


def _get_programs(use_beta, use_beta2):
    key = ("progs", bool(use_beta), bool(use_beta2))
    if key not in _CACHE:
        nc1 = build_l1(use_beta=use_beta)
        nc2 = build_l2(use_beta2=use_beta2)
        _CACHE[key] = (nc1, nc2)
    return _CACHE[key]


def kernel(**inputs):
    from concourse.bass_utils import run_bass_kernel_spmd

    inputs = {k: np.asarray(v) for k, v in inputs.items()}
    use_beta = bool(np.any(np.asarray(inputs["beta1"], np.float32) != 0.0))
    use_beta2 = bool(np.any(np.asarray(inputs["beta2"], np.float32) != 0.0))
    nc1, nc2 = _get_programs(use_beta, use_beta2)
    core_ids = list(range(NCORES))

    r1 = run_bass_kernel_spmd(nc1, prep_l1_inputs(inputs), core_ids)
    # unshard + normalize: rows 0-63 h0 attn, 64 h0 denom, 65-128 h1, 129 h1 denom
    parts = []
    for c in range(NCORES):
        o = np.asarray(r1.results[c]["attn_out"], np.float32)
        parts.append(o[0:64] / o[64:65])
        parts.append(o[65:129] / o[129:130])
    attn_t = np.concatenate(parts, axis=0)

    r2 = run_bass_kernel_spmd(nc2, prep_l2_inputs(inputs, attn_t), core_ids)
    out = np.concatenate(
        [np.asarray(r2.results[c]["out_t"]) for c in range(NCORES)], axis=1)
    return np.ascontiguousarray(out.T.reshape(B, T, C).astype(np.float32))

